# revision 7
# baseline (speedup 1.0000x reference)
"""Absolute sinusoidal positional encoding: out = x + pe[None, :, :].

x: [8, 4096, 1024] f32.  pe[s, 2j] = sin(s / 10000^(2j/D)), pe[s, 2j+1] = cos(...).

Sharding: along sequence across 8 cores. Core k handles x[:, k*512:(k+1)*512, :]
plus the matching 512-row slice of the pe table (computed on host, 2 MiB/core).
Per core the kernel is a pure streaming add: 16 MiB in + 2 MiB pe + 16 MiB out.
"""

import numpy as np

import concourse.bass as bass
import concourse.tile as tile
from concourse import bacc, mybir
from concourse.bass_utils import run_bass_kernel_spmd

B, S, D = 8, 4096, 1024
N_CORES = 8
S_SH = S // N_CORES          # 512 sequence rows per core
ROWS = B * S_SH              # 4096 flat rows per core
P = 128
NBLK = ROWS // P             # 32 row-blocks of 128
PE_BLK = S_SH // P           # 4 pe row-blocks
K = 8                        # row-blocks per tile -> [128, 8, 1024] f32 = 4 MiB
N_ITERS = NBLK // K          # 4

_F32 = mybir.dt.float32
_nc_cache = None


def _build_nc():
    global _nc_cache
    if _nc_cache is not None:
        return _nc_cache
    # Bacc (not raw Bass): its finalize runs generate_event_semaphores,
    # which splits multi-sem waits to satisfy the TRN2 1-wait/inst limit.
    nc = bacc.Bacc("TRN2", target_bir_lowering=False, debug=False,
                   num_devices=N_CORES)
    x_d = nc.declare_dram_parameter("x", [ROWS, D], _F32, isOutput=False)
    pe_d = nc.declare_dram_parameter("pe", [S_SH, D], _F32, isOutput=False)
    out_d = nc.declare_dram_parameter("out", [ROWS, D], _F32, isOutput=True)

    # [p, n, :] = flat row n*128+p. Row r has pe row r mod 512 = (n mod 4)*128+p,
    # so a K=4 block group [4i, 4i+4) pairs elementwise with the whole pe tile.
    xv = x_d[:, :].rearrange("(n p) d -> p n d", p=P)     # [128, 32, 1024]
    ov = out_d[:, :].rearrange("(n p) d -> p n d", p=P)
    pev = pe_d[:, :].rearrange("(m p) d -> p m d", p=P)   # [128, 4, 1024]

    with tile.TileContext(nc) as tc:
        # bufs == N_ITERS so no SBUF slot is ever reused: keeps the
        # sync-wait count per instruction within the ISA limit (walrus
        # rejects TensorTensor with 3 waits) and removes WAR stalls.
        with tc.tile_pool(name="pe", bufs=1) as pe_pool, \
             tc.tile_pool(name="x", bufs=N_ITERS) as x_pool:
            pe_t = pe_pool.tile([P, PE_BLK, D], _F32)
            nc.sync.dma_start(pe_t[:], pev[:])
            for i in range(N_ITERS):
                t = x_pool.tile([P, K, D], _F32)
                nc.sync.dma_start(t[:], xv[:, i * K:(i + 1) * K, :])
                # pe repeats every PE_BLK row-blocks -> one add per repeat
                for r in range(K // PE_BLK):
                    sl = t[:, r * PE_BLK:(r + 1) * PE_BLK, :]
                    nc.vector.tensor_add(sl, sl, pe_t[:])
                nc.scalar.dma_start(ov[:, i * K:(i + 1) * K, :], t[:])
    nc.finalize()
    _nc_cache = nc
    return nc


def _pos_encoding():
    pos = np.arange(S, dtype=np.float32)[:, None]
    j = np.arange(D // 2, dtype=np.float32)[None, :]
    inv_freq = np.power(np.float32(10000.0), np.float32(-2.0) * j / np.float32(D))
    angles = pos * inv_freq
    pe = np.empty((S, D), dtype=np.float32)
    pe[:, 0::2] = np.sin(angles)
    pe[:, 1::2] = np.cos(angles)
    return pe


def _run(x, trace=False):
    x = np.ascontiguousarray(np.asarray(x, dtype=np.float32))
    pe = _pos_encoding()
    nc = _build_nc()
    in_maps = []
    for k in range(N_CORES):
        xk = np.ascontiguousarray(x[:, k * S_SH:(k + 1) * S_SH, :]).reshape(ROWS, D)
        pek = np.ascontiguousarray(pe[k * S_SH:(k + 1) * S_SH, :])
        in_maps.append({"x": xk, "pe": pek})
    res = run_bass_kernel_spmd(nc, in_maps, list(range(N_CORES)), trace=trace)
    outs = [res.results[k]["out"].reshape(B, S_SH, D) for k in range(N_CORES)]
    full = np.concatenate(outs, axis=1)
    return full, res


def kernel(x):
    return _run(x, trace=False)[0]


# revision 10
# speedup vs baseline: 1.0395x; 1.0395x over previous
"""Absolute sinusoidal positional encoding: out = x + pe[None, :, :].

x: [8, 4096, 1024] f32.  pe[s, 2j] = sin(s / 10000^(2j/D)), pe[s, 2j+1] = cos(...).

Sharding: along sequence across 8 cores. Core k handles x[:, k*512:(k+1)*512, :]
plus the matching 512-row slice of the pe table (computed on host, 2 MiB/core).
Per-core kernel is a pure streaming add: 16 MiB in + 2 MiB pe + 16 MiB out,
HBM-bandwidth bound.
"""

import numpy as np

import concourse.bass as bass
import concourse.tile as tile
from concourse import bacc, mybir
from concourse.bass_utils import run_bass_kernel_spmd

B, S, D = 8, 4096, 1024
N_CORES = 8
S_SH = S // N_CORES          # 512 sequence rows per core
ROWS = B * S_SH              # 4096 flat rows per core
P = 128
NBLK = ROWS // P             # 32 row-blocks of 128
PE_BLK = S_SH // P           # 4 pe row-blocks

# row-blocks per tile (tile bytes = K * 512 KiB); last tiles can be smaller
K = 2
_F32 = mybir.dt.float32
_nc_cache = None


def _build_nc():
    global _nc_cache
    if _nc_cache is not None:
        return _nc_cache
    # Bacc (not raw Bass): its finalize runs generate_event_semaphores,
    # which splits multi-sem waits to satisfy the TRN2 1-wait/inst limit.
    nc = bacc.Bacc("TRN2", target_bir_lowering=False, debug=False,
                   num_devices=N_CORES)
    x_d = nc.declare_dram_parameter("x", [ROWS, D], _F32, isOutput=False)
    pe_d = nc.declare_dram_parameter("pe", [S_SH, D], _F32, isOutput=False)
    out_d = nc.declare_dram_parameter("out", [ROWS, D], _F32, isOutput=True)

    # [p, n, :] = flat row n*128+p. Row r has pe row r mod 512 = (n mod 4)*128+p,
    # so row-block n pairs with pe row-block (n mod 4).
    xv = x_d[:, :].rearrange("(n p) d -> p n d", p=P)     # [128, 32, 1024]
    ov = out_d[:, :].rearrange("(n p) d -> p n d", p=P)
    pev = pe_d[:, :].rearrange("(m p) d -> p m d", p=P)   # [128, 4, 1024]

    # tile sizes in row-blocks; shrink the final tiles to cut the tail
    # (last add + last store sit on the critical path after the last load)
    sizes = [K] * (NBLK // K)
    if K >= 2:
        sizes = sizes[:-1] + [1, 1]
    assert sum(sizes) == NBLK

    with tile.TileContext(nc) as tc:
        # one slot per tile: no SBUF slot reuse -> no WAR waits
        with tc.tile_pool(name="pe", bufs=1) as pe_pool, \
             tc.tile_pool(name="x", bufs=len(sizes)) as x_pool:
            pe_t = pe_pool.tile([P, PE_BLK, D], _F32)
            nc.sync.dma_start(pe_t[:], pev[:])
            n0 = 0
            for i, sz in enumerate(sizes):
                t = x_pool.tile([P, sz, D], _F32, name="t", tag="t")
                # alternate load issue ring: Sync (HWDGE) / GpSimd (SWDGE)
                ld_eng = nc.sync if i % 2 == 0 else nc.gpsimd
                ld_eng.dma_start(t[:], xv[:, n0:n0 + sz, :])
                r = 0
                while r < sz:
                    m = (n0 + r) % PE_BLK
                    c = min(sz - r, PE_BLK - m)
                    sl = t[:, r:r + c, :]
                    nc.vector.tensor_add(sl, sl, pe_t[:, m:m + c, :])
                    r += c
                nc.scalar.dma_start(ov[:, n0:n0 + sz, :], t[:])
                n0 += sz
    nc.finalize()
    _nc_cache = nc
    return nc


def _pos_encoding():
    """pe table, replicating reference's fp32 jax computation. Use jax when
    importable so the values match the reference bit-for-bit on the same
    backend; fall back to a float32 numpy pipeline (~1e-7 off per element,
    worst-case ~4e-4 after the pos*inv_freq f32 rounding amplification)."""
    try:
        import jax
        import jax.numpy as jnp

        pos = jnp.arange(S, dtype=jnp.float32)[:, None]
        j = jnp.arange(D // 2, dtype=jnp.float32)[None, :]
        inv_freq = jnp.power(10000.0, -2.0 * j / D)
        angles = pos * inv_freq
        pe = jnp.empty((S, D), dtype=jnp.float32)
        pe = pe.at[:, 0::2].set(jnp.sin(angles))
        pe = pe.at[:, 1::2].set(jnp.cos(angles))
        return np.asarray(pe, dtype=np.float32)
    except Exception:
        pos = np.arange(S, dtype=np.float32)[:, None]
        j = np.arange(D // 2, dtype=np.float32)[None, :]
        inv_freq = np.power(np.float32(10000.0),
                            np.float32(-2.0) * j / np.float32(D))
        angles = pos * inv_freq
        pe = np.empty((S, D), dtype=np.float32)
        pe[:, 0::2] = np.sin(angles)
        pe[:, 1::2] = np.cos(angles)
        return pe


def _run(x, trace=False):
    x = np.ascontiguousarray(np.asarray(x, dtype=np.float32))
    pe = _pos_encoding()
    nc = _build_nc()
    in_maps = []
    for k in range(N_CORES):
        xk = np.ascontiguousarray(x[:, k * S_SH:(k + 1) * S_SH, :]).reshape(ROWS, D)
        pek = np.ascontiguousarray(pe[k * S_SH:(k + 1) * S_SH, :])
        in_maps.append({"x": xk, "pe": pek})
    res = run_bass_kernel_spmd(nc, in_maps, list(range(N_CORES)), trace=trace)
    outs = [res.results[k]["out"].reshape(B, S_SH, D) for k in range(N_CORES)]
    full = np.concatenate(outs, axis=1)
    return full, res


def kernel(x):
    return _run(x, trace=False)[0]
